# revision 11
# baseline (speedup 1.0000x reference)
"""Fused cross-attention kernel for Trainium2, 8-way data-parallel over batch.

Per core (one batch element):
  QT[d, hw] = (Wq @ Jp + bq)        via lhsT=[WqT; bq], rhs=[Jp; 1]
  K [d, hw] = (Wk @ Jg + bk)
  V [hw, d] = (Jg.T @ WvT + bv)     stored with 2 ones cols -> [V | 1 | 1]
  For each q-block (512 queries):
    for each k-chunk (128 keys):
      S^T[k, q]   = K-chunk.T @ QT          (PSUM, 2 matmuls over d-halves)
      E^T         = exp(S^T / 16)           (scalar engine, PSUM -> SBUF)
      O[q, 258]  += E^T-subtile.T @ [V|1|1] (PSUM accum; col 256 = softmax sum)
    out[q, d] = O[:, :256] * (1 / O[:, 256])

All matmuls run in float32r (TF32-like full-rate fp32 mode). fp32r ISA
restrictions honored: even innermost dst size, dst start_partition 0, inputs
materialized as float32r by their producers (DRAM tensors declared float32r).
Q/K/V live in per-block tiles so attention can overlap the projection tail.
"""

import sys

sys.path.insert(0, "/opt/trn_rl_repo")

import numpy as np

import concourse.bacc as bacc
import concourse.mybir as mybir
import concourse.tile as tile
from concourse.bass_utils import run_bass_kernel_spmd

B, C, H, W = 8, 64, 64, 64
HW = H * W  # 4096
D = 256
CE = C + 1  # channels + ones row for bias folding
N_CORES = 8
QB = 512  # queries per block
N_QB = HW // QB  # 8
N_KC = HW // 128  # 32 key chunks
DV = D + 2  # V row width: 256 values + 2 ones columns (fp32r needs even dst)
F32 = mybir.dt.float32
MM_DT = mybir.dt.float32r  # full-rate fp32 matmul mode (TF32-like)

_CACHE = {}


def build_module(
    reps: int = 1,
    st_bufs: int = 3,
    op_bufs: int = 5,
    ep_bufs: int = 3,
    split: int = 1,
    pp_bufs: int = 4,
    dtype: str = "f32r",
    exp_split: int = 1,
    ck_pair: bool = False,
):
    mm_dt = MM_DT if dtype == "f32r" else mybir.dt.bfloat16
    nc = bacc.Bacc("TRN2", target_bir_lowering=False)
    jp_d = nc.dram_tensor("jp", [CE, HW], mm_dt, kind="ExternalInput")
    jg_d = nc.dram_tensor("jg", [CE, HW], mm_dt, kind="ExternalInput")
    wq_d = nc.dram_tensor("wq", [CE, D], mm_dt, kind="ExternalInput")
    wk_d = nc.dram_tensor("wk", [CE, D], mm_dt, kind="ExternalInput")
    wv_d = nc.dram_tensor("wv", [CE, D], mm_dt, kind="ExternalInput")
    ones_d = nc.dram_tensor("ones", [128, N_KC, 2], mm_dt, kind="ExternalInput")
    out_d = nc.dram_tensor("out", [HW, D], F32, kind="ExternalOutput")

    with tile.TileContext(nc) as tc:
        with tc.tile_pool(name="const", bufs=1) as const:
            jp_t = const.tile([CE, HW], mm_dt, tag="jp")
            jg_t = const.tile([CE, HW], mm_dt, tag="jg")
            wq_t = const.tile([CE, D], mm_dt, tag="wq")
            wk_t = const.tile([CE, D], mm_dt, tag="wk")
            wv_t = const.tile([CE, D], mm_dt, tag="wv")
            # per-block tiles for fine-grained dependencies
            qt_b = [
                const.tile([128, 2, QB], mm_dt, tag=f"qt{g}", name=f"qt_{g}")
                for g in range(N_QB)
            ]
            kt_g = [
                const.tile([128, 2, QB], mm_dt, tag=f"kt{g}", name=f"kt_{g}")
                for g in range(N_QB)
            ]
            vt_g = [
                const.tile([128, 4, DV], mm_dt, tag=f"vt{g}", name=f"vt_{g}")
                for g in range(N_QB)
            ]

            nc.sync.dma_start(wq_t[:], wq_d[:])
            nc.sync.dma_start(wk_t[:], wk_d[:])
            nc.sync.dma_start(wv_t[:], wv_d[:])
            for g in range(N_QB):
                hs = slice(g * QB, (g + 1) * QB)
                nc.sync.dma_start(jg_t[:, hs], jg_d[:, hs])
                nc.sync.dma_start(jp_t[:, hs], jp_d[:, hs])
                nc.sync.dma_start(vt_g[g][:, :, D:DV], ones_d[:, 4 * g : 4 * g + 4, :])

            for _rep in range(reps):
                # ---- projections ----
                # Order: Q(0) first (attention qb=0 needs it), then K/V in
                # ascending k-chunk order so attention consumes them streaming,
                # remaining Q blocks at the end.
                with tc.tile_pool(name="pp", bufs=pp_bufs, space="PSUM") as pp:

                    def proj_q(g):
                        hs = slice(g * QB, (g + 1) * QB)
                        for dh in range(2):
                            ds = slice(dh * 128, (dh + 1) * 128)
                            psq = pp.tile([128, QB], F32, tag="proj")
                            nc.tensor.matmul(psq[:], wq_t[:, ds], jp_t[:, hs])
                            nc.vector.tensor_copy(qt_b[g][:, dh, :], psq[:])

                    proj_q(0)
                    for g in range(N_QB):
                        hs = slice(g * QB, (g + 1) * QB)
                        for dh in range(2):
                            ds = slice(dh * 128, (dh + 1) * 128)
                            psk = pp.tile([128, QB], F32, tag="proj")
                            nc.tensor.matmul(psk[:], wk_t[:, ds], jg_t[:, hs])
                            nc.vector.tensor_copy(kt_g[g][:, dh, :], psk[:])
                        for j in range(4):
                            ck = 4 * g + j
                            ks = slice(ck * 128, (ck + 1) * 128)
                            psv = pp.tile([128, D], F32, tag="projv")
                            nc.tensor.matmul(psv[:], jg_t[:, ks], wv_t[:])
                            nc.vector.tensor_copy(vt_g[g][:, j, :D], psv[:])
                    for g in range(1, N_QB):
                        proj_q(g)

                # ---- attention ----
                SW = QB // split  # S^T / exp tile width
                QS_PER = SW // 128  # q-subtiles per exp tile
                with (
                    tc.tile_pool(name="stp", bufs=st_bufs, space="PSUM") as stp,
                    tc.tile_pool(name="op", bufs=op_bufs, space="PSUM") as op,
                    tc.tile_pool(name="ep", bufs=ep_bufs) as ep,
                    tc.tile_pool(name="outp", bufs=3) as outp,
                    tc.tile_pool(name="lp", bufs=4) as lp,
                ):
                    for qb in range(N_QB):
                        o_ps = [
                            op.tile([128, DV], F32, tag="o", name=f"o_{qb}_{i}")
                            for i in range(4)
                        ]
                        if ck_pair:
                            for cp in range(N_KC // 2):
                                st2 = stp.tile([128, 2, QB], F32, tag="st")
                                for c in range(2):
                                    ck = 2 * cp + c
                                    g, j = ck // 4, ck % 4
                                    for dh in range(2):
                                        nc.tensor.matmul(
                                            st2[:, c, :],
                                            kt_g[g][:, dh, j * 128 : (j + 1) * 128],
                                            qt_b[qb][:, dh, :],
                                            start=(dh == 0),
                                            stop=(dh == 1),
                                        )
                                et2 = ep.tile([128, 2 * QB], mm_dt, tag="e")
                                nc.scalar.activation(
                                    et2[:],
                                    st2[:, :, :],
                                    mybir.ActivationFunctionType.Exp,
                                    scale=1.0 / 16.0,
                                )
                                for c in range(2):
                                    ck = 2 * cp + c
                                    g, j = ck // 4, ck % 4
                                    for i in range(4):
                                        nc.tensor.matmul(
                                            o_ps[i][:],
                                            et2[
                                                :,
                                                c * QB + i * 128 : c * QB + (i + 1) * 128,
                                            ],
                                            vt_g[g][:, j, :],
                                            start=(ck == 0),
                                            stop=(ck == N_KC - 1),
                                        )
                            for qsub in range(4):
                                row = qb * 4 + qsub
                                linv = lp.tile([128, 1], F32, tag="l")
                                nc.vector.reciprocal(linv[:], o_ps[qsub][:, D : D + 1])
                                ot = outp.tile([128, D], F32, tag="ot")
                                nc.vector.tensor_scalar_mul(
                                    ot[:], o_ps[qsub][:, :D], linv[:]
                                )
                                nc.sync.dma_start(
                                    out_d[row * 128 : (row + 1) * 128, :], ot[:]
                                )
                            continue
                        for ck in range(N_KC):
                            g, j = ck // 4, ck % 4
                            for sp in range(split):
                                qlo = sp * SW
                                st = stp.tile([128, SW], F32, tag="st")
                                for dh in range(2):
                                    nc.tensor.matmul(
                                        st[:],
                                        kt_g[g][:, dh, j * 128 : (j + 1) * 128],
                                        qt_b[qb][:, dh, qlo : qlo + SW],
                                        start=(dh == 0),
                                        stop=(dh == 1),
                                    )
                                EW = SW // exp_split
                                EQ = EW // 128
                                for es in range(exp_split):
                                    et = ep.tile([128, EW], mm_dt, tag="e")
                                    nc.scalar.activation(
                                        et[:],
                                        st[:, es * EW : (es + 1) * EW],
                                        mybir.ActivationFunctionType.Exp,
                                        scale=1.0 / 16.0,
                                    )
                                    for i in range(EQ):
                                        qsub = sp * QS_PER + es * EQ + i
                                        nc.tensor.matmul(
                                            o_ps[qsub][:],
                                            et[:, i * 128 : (i + 1) * 128],
                                            vt_g[g][:, j, :],
                                            start=(ck == 0),
                                            stop=(ck == N_KC - 1),
                                        )
                        for qsub in range(4):
                            row = qb * 4 + qsub
                            linv = lp.tile([128, 1], F32, tag="l")
                            nc.vector.reciprocal(linv[:], o_ps[qsub][:, D : D + 1])
                            ot = outp.tile([128, D], F32, tag="ot")
                            nc.vector.tensor_scalar_mul(
                                ot[:], o_ps[qsub][:, :D], linv[:]
                            )
                            nc.sync.dma_start(
                                out_d[row * 128 : (row + 1) * 128, :], ot[:]
                            )

    nc.compile()
    return nc


def _get_module(reps: int = 1, **kw):
    key = (reps, tuple(sorted(kw.items())))
    if key not in _CACHE:
        _CACHE[key] = build_module(reps, **kw)
    return _CACHE[key]


_ONES = np.ones((128, N_KC, 2), np.float32)
_ROW1 = np.ones((1, HW), np.float32)


def _prep_in_maps(inputs, dtype="f32r"):
    import ml_dtypes

    npdt = np.float32 if dtype == "f32r" else ml_dtypes.bfloat16
    jp = np.asarray(inputs["Jp_embedding"], np.float32).reshape(B, C, HW)
    jg = np.asarray(inputs["Jg_embedding"], np.float32).reshape(B, C, HW)
    wq = np.concatenate(
        [
            np.asarray(inputs["Wq"], np.float32).T,
            np.asarray(inputs["bq"], np.float32)[None, :],
        ],
        0,
    )
    wk = np.concatenate(
        [
            np.asarray(inputs["Wk"], np.float32).T,
            np.asarray(inputs["bk"], np.float32)[None, :],
        ],
        0,
    )
    wv = np.concatenate(
        [
            np.asarray(inputs["Wv"], np.float32).T,
            np.asarray(inputs["bv"], np.float32)[None, :],
        ],
        0,
    )
    return [
        {
            "jp": np.concatenate([jp[b], _ROW1], 0).astype(npdt),
            "jg": np.concatenate([jg[b], _ROW1], 0).astype(npdt),
            "wq": wq.astype(npdt),
            "wk": wk.astype(npdt),
            "wv": wv.astype(npdt),
            "ones": _ONES.astype(npdt),
        }
        for b in range(B)
    ]


def kernel(**inputs):
    nc = _get_module()
    in_maps = _prep_in_maps(inputs)
    res = run_bass_kernel_spmd(nc, in_maps, core_ids=list(range(N_CORES)))
    return np.stack(
        [res.results[b]["out"].reshape(D, H, W) for b in range(B)], axis=0
    )
